# revision 25
# baseline (speedup 1.0000x reference)
"""Distributed Trainium2 kernel for nn_Attention (B=2, N=4096, C=512, H=8).

Sharding: 8 cores = (batch in {0,1}) x (head-pair in {0..3}).
Each core computes QKV projection for its 2 heads, full NxN attention for
those heads, and the partial output projection (its 2 heads' rows of
w_proj).  The host sums the 4 partials per batch and adds b_proj.

Device dataflow (all-transposed, zero on-device attention transposes):
  - host passes x[b].T (bf16) so activations arrive contraction-major
  - qT/kT [64d x N] per head (d on partitions); V in natural [k x d]
    layout augmented with a ones column (denominator for free)
  - scores computed transposed: sT[k,q] = K @ qT  (two heads row-packed
    on the PE, K=64 each)
  - exp on ScalarE (scale=HD^-0.5 folded in), bf16 out; the exp'd tile is
    exactly the stationary operand A.V needs
  - AV: outT[d(+1),q] += V_aug.T @ attn, accumulated over 32 k-chunks
  - normalize: reciprocal of the denom row, partition_broadcast (gpsimd),
    one DVE multiply; per-head projection row-packed (K=64+64) into one
    PSUM bank; DVE copy, DMA out.
"""

import numpy as np
import ml_dtypes

B, N, C = 2, 4096, 512
H = 8
HD = C // H           # 64
SCALE = HD ** -0.5
NCORES = 8
RG = N // 512         # 8 row groups of 512
QT = N // 512         # 8 q tiles of 512
KC = N // 128         # 32 k chunks of 128

_CACHE = {}


def _pbcast(ap, nparts):
    """AP view that reads partition 0 of `ap` broadcast over nparts partitions."""
    import concourse.bass as bass
    return bass.AP(tensor=ap.tensor, offset=ap.offset, ap=[[0, nparts]] + list(ap.ap[1:]))


def _build_bass(loop_reps=None):
    import concourse.bass as bass
    import concourse.mybir as mybir
    import concourse.tile as tile
    from concourse import bacc

    f32 = mybir.dt.float32
    bf16 = mybir.dt.bfloat16
    i16 = mybir.dt.int16
    Exp = mybir.ActivationFunctionType.Exp
    # Schraudolph exp -> bf16 bits on DVE: bits16 = A*s + B (f32->int16
    # convert rounds to nearest), bitcast to bf16.  Offloads ~1/3 of the
    # softmax exp work from the ScalarE (the bottleneck) to the DVE.
    SCHR_A = float(128.0 * np.log2(np.e) * SCALE)
    SCHR_B = float(127.0 * 128.0 - 7.4)

    nc = bacc.Bacc()

    xt_d = nc.dram_tensor("xt", [C, N], bf16, kind="ExternalInput")
    wq_d = nc.dram_tensor("wq", [C, 2 * HD], bf16, kind="ExternalInput")
    wk_d = nc.dram_tensor("wk", [C, 2 * HD], bf16, kind="ExternalInput")
    wv_d = nc.dram_tensor("wv", [C, 2 * HD], bf16, kind="ExternalInput")
    wp_d = nc.dram_tensor("wp", [2 * HD, C], bf16, kind="ExternalInput")
    out_d = nc.dram_tensor("out", [N, C], bf16, kind="ExternalOutput")

    with tile.TileContext(nc) as tc:
        from concourse import library_config
        nc.gpsimd.load_library(library_config.proxy)
        with (
            tc.tile_pool(name="persist", bufs=1) as persist,
            tc.tile_pool(name="attn", bufs=6) as apool,
            tc.tile_pool(name="norm", bufs=2) as npool,
            tc.tile_pool(name="small", bufs=4) as small,
            tc.tile_pool(name="y", bufs=3) as ypool,
            tc.tile_pool(name="spsum", bufs=2, space="PSUM") as spool,
            tc.tile_pool(name="avpsum", bufs=4, space="PSUM") as avpool,
        ):
            # ---- weights ----
            wq_sb = persist.tile([128, 4, 2 * HD], bf16, tag="wq")
            wk_sb = persist.tile([128, 4, 2 * HD], bf16, tag="wk")
            wv_sb = persist.tile([128, 4, 2 * HD], bf16, tag="wv")
            for ctr in range(4):
                nc.sync.dma_start(wq_sb[:, ctr, :], wq_d[ctr * 128:(ctr + 1) * 128, :])
                nc.sync.dma_start(wk_sb[:, ctr, :], wk_d[ctr * 128:(ctr + 1) * 128, :])
                nc.sync.dma_start(wv_sb[:, ctr, :], wv_d[ctr * 128:(ctr + 1) * 128, :])
            wp_sb = persist.tile([128, C], bf16, tag="wp")
            nc.sync.dma_start(wp_sb[:, :], wp_d[:, :])
            ones64 = persist.tile([1, HD], f32, tag="ones64")
            nc.vector.memset(ones64[:, :], 1.0)

            # ---- persistent per-row-group tiles ----
            xsb = [persist.tile([128, 4, 512], bf16, tag=f"xsb{rg}", name=f"xsb{rg}") for rg in range(RG)]
            qTt = [persist.tile([128, 512], bf16, tag=f"qT{rg}", name=f"qT{rg}") for rg in range(RG)]
            kTt = [persist.tile([128, 512], bf16, tag=f"kT{rg}", name=f"kT{rg}") for rg in range(RG)]
            vt = [persist.tile([128, 2, 4, HD + 1], bf16, tag=f"v{rg}", name=f"v{rg}") for rg in range(RG)]

            def prologue(rg):
                # DMA x[b].T chunk: [512 C x 512 rows] -> 4 partition chunks
                for ctr in range(4):
                    nc.sync.dma_start(
                        xsb[rg][:, ctr, :],
                        xt_d[ctr * 128:(ctr + 1) * 128, rg * 512:(rg + 1) * 512],
                    )
                # qT/kT: [c_out(2 heads x 64) x rows]  = w.T @ x.T-chunk
                ps_q = avpool.tile([128, 512], f32, tag="av")
                ps_k = avpool.tile([128, 512], f32, tag="av")
                for ctr in range(4):
                    nc.tensor.matmul(
                        ps_q[:, :], wq_sb[:, ctr, :], xsb[rg][:, ctr, :],
                        start=(ctr == 0), stop=(ctr == 3),
                    )
                for ctr in range(4):
                    nc.tensor.matmul(
                        ps_k[:, :], wk_sb[:, ctr, :], xsb[rg][:, ctr, :],
                        start=(ctr == 0), stop=(ctr == 3),
                    )
                nc.scalar.copy(qTt[rg][:, :], ps_q[:, :])
                nc.scalar.copy(kTt[rg][:, :], ps_k[:, :])
                # V natural layout: [rows x c_out]; rows on partitions
                ps_v = avpool.tile([128, 4, 128], f32, tag="av")
                for rcl in range(4):
                    for ctr in range(4):
                        nc.tensor.matmul(
                            ps_v[:, rcl, :],
                            xsb[rg][:, ctr, rcl * 128:(rcl + 1) * 128],
                            wv_sb[:, ctr, :],
                            start=(ctr == 0), stop=(ctr == 3),
                        )
                # scatter into v tiles: [128k x (h, rcl, d)] ; ones column
                nc.scalar.copy(
                    vt[rg][:, :, :, 0:HD],
                    ps_v.rearrange("p rcl (h d) -> p h rcl d", h=2),
                )
                nc.vector.memset(vt[rg][:, :, :, HD:HD + 1], 1.0)

            def qk_exp(qt, kc, use_act):
                rgk, kcl = divmod(kc, 4)
                sco = spool.tile([128, 2, 512], f32, tag="scores")
                # scores.T [k x q], two heads row-packed (K=64 each)
                nc.tensor.matmul(
                    sco[:, 0, :],
                    kTt[rgk][0:HD, kcl * 128:(kcl + 1) * 128],
                    qTt[qt][0:HD, :],
                    start=True, stop=True, skip_group_check=True,
                )
                nc.tensor.matmul(
                    sco[:, 1, :],
                    kTt[rgk][HD:2 * HD, kcl * 128:(kcl + 1) * 128],
                    qTt[qt][HD:2 * HD, :],
                    start=True, stop=True, skip_group_check=True,
                )
                att = apool.tile([128, 2, 512], bf16, tag="attn")
                if use_act:
                    nc.scalar.activation(att[:, :, :], sco[:, :, :], Exp, scale=SCALE)
                else:
                    nc.vector.tensor_scalar(
                        att.bitcast(i16)[:, :, :], sco[:, :, :], SCHR_A, SCHR_B,
                        op0=mybir.AluOpType.mult, op1=mybir.AluOpType.add,
                    )
                return att

            def av_acc(qt, kc, att, av0, av1, first, last):
                rgk, kcl = divmod(kc, 4)
                nc.tensor.matmul(
                    av0[:, :], vt[rgk][:, 0, kcl, :], att[:, 0, :],
                    start=first, stop=last, skip_group_check=True,
                )
                nc.tensor.matmul(
                    av1[:, :], vt[rgk][:, 1, kcl, :], att[:, 1, :],
                    start=first, stop=last, skip_group_check=True,
                )

            def post_stage1(qt, av0, av1):
                # move AV out of PSUM on the ScalarE (frees the av psum slots)
                avsb = npool.tile([128, 2, 512], f32, tag="avsb", name=f"avsb{qt}")
                nc.scalar.copy(avsb[0:HD + 1, 0, :], av0[:, :])
                nc.scalar.copy(avsb[0:HD + 1, 1, :], av1[:, :])
                return avsb

            def post_stage1b(qt, avsb):
                rec0 = small.tile([1, 512], f32, tag="rec0", name=f"rec0_{qt}")
                rec1 = small.tile([1, 512], f32, tag="rec1", name=f"rec1_{qt}")
                nc.vector.reciprocal(rec0[:, :], avsb[HD:HD + 1, 0, :])
                nc.vector.reciprocal(rec1[:, :], avsb[HD:HD + 1, 1, :])
                return avsb, rec0, rec1

            def post_stage2(qt, avsb, rec0, rec1):
                rbc0 = small.tile([HD, 512], f32, tag="rbc0", name=f"rbc0_{qt}")
                rbc1 = small.tile([HD, 512], f32, tag="rbc1", name=f"rbc1_{qt}")
                nc.gpsimd.partition_broadcast(rbc0[:, :], rec0[:, :])
                nc.gpsimd.partition_broadcast(rbc1[:, :], rec1[:, :])
                # normalized attention output, both heads stacked [128 x 512q]
                avn = npool.tile([128, 512], bf16, tag="avn", name=f"avn{qt}")
                nc.gpsimd.tensor_mul(avn[0:HD, :], avsb[0:HD, 0, :], rbc0[:, :])
                nc.gpsimd.tensor_mul(avn[HD:2 * HD, :], avsb[0:HD, 1, :], rbc1[:, :])
                return avn

            def post_stage3(qt, avn):
                for qc in range(4):
                    ps_y = avpool.tile([128, 512], f32, tag="av", name=f"psy{qt}_{qc}")
                    nc.tensor.matmul(
                        ps_y[:, :], avn[:, qc * 128:(qc + 1) * 128],
                        wp_sb[:, :], start=True, stop=True,
                        skip_group_check=True,
                    )
                    y_sb = ypool.tile([128, 512], bf16, tag="y", name=f"y{qt}_{qc}")
                    nc.scalar.copy(y_sb[:, :], ps_y[:, :])
                    nc.sync.dma_start(
                        out_d[qt * 512 + qc * 128: qt * 512 + (qc + 1) * 128, :],
                        y_sb[:, :],
                    )

            # ---- emission: weave prologue with qt0's k-sweep ----
            def emit_body():
                avs = {}
                pend = {}
                stage_q = []

                def push(qt, kc, use_act):
                    att = qk_exp(qt, kc, use_act)
                    if pend.get(qt) is not None:
                        pkc, patt = pend[qt]
                        av_acc(qt, pkc, patt, *avs[qt], pkc == 0, pkc == KC - 1)
                    pend[qt] = (kc, att)

                def drain_av(qt):
                    pkc, patt = pend.pop(qt)
                    av_acc(qt, pkc, patt, *avs[qt], pkc == 0, pkc == KC - 1)

                for rg in range(RG):
                    prologue(rg)
                for pair in range(QT // 2):
                    qtA, qtB = 2 * pair, 2 * pair + 1
                    avs[qtA] = (avpool.tile([HD + 1, 512], f32, tag="av", name=f"av0_t{qtA}"),
                                avpool.tile([HD + 1, 512], f32, tag="av", name=f"av1_t{qtA}"))
                    avs[qtB] = (avpool.tile([HD + 1, 512], f32, tag="av", name=f"av0_t{qtB}"),
                                avpool.tile([HD + 1, 512], f32, tag="av", name=f"av1_t{qtB}"))
                    for kc in range(KC):
                        push(qtA, kc, kc < KC - 4)
                        push(qtB, kc, False)
                        if kc == 2 and stage_q:
                            stage_q.pop(0)()
                        if kc == 6 and stage_q:
                            stage_q.pop(0)()
                    drain_av(qtA)
                    drain_av(qtB)
                    import os as _os
                    if _os.environ.get("KERNEL_NOPOST"):
                        for qt in (qtA, qtB):
                            avsb = npool.tile([128, 2, 512], f32, tag="avsb", name=f"avsb{qt}")
                            a0, a1 = avs.pop(qt)
                            nc.scalar.copy(avsb[0:HD + 1, 0, :], a0[:, :])
                            nc.scalar.copy(avsb[0:HD + 1, 1, :], a1[:, :])
                            nc.sync.dma_start(out_d[qt * 512:qt * 512 + 128, :],
                                              avsb[:, 0, :])
                        continue
                    sbA = post_stage1(qtA, *avs.pop(qtA))
                    sbB = post_stage1(qtB, *avs.pop(qtB))
                    # previous pair's projection now that av slots cycled
                    if stage_q:
                        stage_q.pop(0)()

                    def mk1b(qa, sa, qb, sb_):
                        def s1b():
                            ca = post_stage1b(qa, sa)
                            cb = post_stage1b(qb, sb_)

                            def s2():
                                avn_a = post_stage2(qa, *ca)
                                avn_b = post_stage2(qb, *cb)
                                stage_q.append(lambda: (post_stage3(qa, avn_a),
                                                        post_stage3(qb, avn_b)))
                            stage_q.insert(0, s2)
                        return s1b

                    stage_q.append(mk1b(qtA, sbA, qtB, sbB))
                while stage_q:
                    stage_q.pop(0)()

            if loop_reps:
                with tc.For_i(0, loop_reps, 1):
                    emit_body()
            else:
                emit_body()

    nc.compile()
    return nc


def _get_nc():
    if "nc" not in _CACHE:
        _CACHE["nc"] = _build_bass()
    return _CACHE["nc"]


def _make_in_maps(x, w_qkv, w_proj):
    bf = ml_dtypes.bfloat16
    in_maps = []
    for core in range(NCORES):
        b, j = divmod(core, 4)
        xt = np.ascontiguousarray(x[b].T).astype(bf)            # [C, N]
        wq = np.ascontiguousarray(w_qkv[:, 128 * j:128 * j + 128]).astype(bf)
        wk = np.ascontiguousarray(w_qkv[:, C + 128 * j:C + 128 * j + 128]).astype(bf)
        wv = np.ascontiguousarray(w_qkv[:, 2 * C + 128 * j:2 * C + 128 * j + 128]).astype(bf)
        wp = np.ascontiguousarray(w_proj[128 * j:128 * j + 128, :]).astype(bf)
        in_maps.append({"xt": xt, "wq": wq, "wk": wk, "wv": wv, "wp": wp})
    return in_maps


def _run(x, w_qkv, w_proj, b_proj, trace=False):
    from concourse.bass_utils import run_bass_kernel_spmd

    nc = _get_nc()
    in_maps = _make_in_maps(x, w_qkv, w_proj)
    res = run_bass_kernel_spmd(nc, in_maps, core_ids=list(range(NCORES)), trace=trace)
    out = np.zeros((B, N, C), dtype=np.float32)
    for core in range(NCORES):
        b = core // 4
        out[b] += res.results[core]["out"].astype(np.float32)
    out += b_proj.astype(np.float32)
    return out, res


def kernel(x, w_qkv, w_proj, b_proj):
    x = np.asarray(x, dtype=np.float32)
    w_qkv = np.asarray(w_qkv, dtype=np.float32)
    w_proj = np.asarray(w_proj, dtype=np.float32)
    b_proj = np.asarray(b_proj, dtype=np.float32)
    out, _ = _run(x, w_qkv, w_proj, b_proj, trace=False)
    return out


# revision 26
# speedup vs baseline: 1.2305x; 1.2305x over previous
"""Distributed Trainium2 kernel for nn_Attention (B=2, N=4096, C=512, H=8).

Sharding: 8 cores = (batch in {0,1}) x (head-pair in {0..3}).
Each core computes QKV projection for its 2 heads, full NxN attention for
those heads, and the partial output projection (its 2 heads' rows of
w_proj).  The host sums the 4 partials per batch and adds b_proj.

Device dataflow (all-transposed, zero on-device attention transposes):
  - host passes x[b].T (bf16) so activations arrive contraction-major
  - qT/kT [64d x N] per head (d on partitions); V in natural [k x d]
    layout augmented with a ones column (denominator for free)
  - scores computed transposed: sT[k,q] = K @ qT  (two heads row-packed
    on the PE, K=64 each)
  - exp on ScalarE (scale=HD^-0.5 folded in), bf16 out; the exp'd tile is
    exactly the stationary operand A.V needs
  - AV: outT[d(+1),q] += V_aug.T @ attn, accumulated over 32 k-chunks
  - normalize: reciprocal of the denom row, partition_broadcast (gpsimd),
    one DVE multiply; per-head projection row-packed (K=64+64) into one
    PSUM bank; DVE copy, DMA out.
"""

import numpy as np
import ml_dtypes

B, N, C = 2, 4096, 512
H = 8
HD = C // H           # 64
SCALE = HD ** -0.5
NCORES = 8
RG = N // 512         # 8 row groups of 512
QT = N // 512         # 8 q tiles of 512
KC = N // 128         # 32 k chunks of 128

_CACHE = {}


def _pbcast(ap, nparts):
    """AP view that reads partition 0 of `ap` broadcast over nparts partitions."""
    import concourse.bass as bass
    return bass.AP(tensor=ap.tensor, offset=ap.offset, ap=[[0, nparts]] + list(ap.ap[1:]))


def _build_bass(loop_reps=None):
    import concourse.bass as bass
    import concourse.mybir as mybir
    import concourse.tile as tile
    from concourse import bacc

    f32 = mybir.dt.float32
    bf16 = mybir.dt.bfloat16
    i16 = mybir.dt.int16
    Exp = mybir.ActivationFunctionType.Exp
    # Schraudolph exp -> bf16 bits on DVE: bits16 = A*s + B (f32->int16
    # convert rounds to nearest), bitcast to bf16.  Offloads ~1/3 of the
    # softmax exp work from the ScalarE (the bottleneck) to the DVE.
    SCHR_A = float(128.0 * np.log2(np.e) * SCALE)
    SCHR_B = float(127.0 * 128.0 - 7.4)

    nc = bacc.Bacc()

    xt_d = nc.dram_tensor("xt", [C, N], bf16, kind="ExternalInput")
    wq_d = nc.dram_tensor("wq", [C, 2 * HD], bf16, kind="ExternalInput")
    wk_d = nc.dram_tensor("wk", [C, 2 * HD], bf16, kind="ExternalInput")
    wv_d = nc.dram_tensor("wv", [C, 2 * HD], bf16, kind="ExternalInput")
    wp_d = nc.dram_tensor("wp", [2 * HD, C], bf16, kind="ExternalInput")
    out_d = nc.dram_tensor("out", [N, C], bf16, kind="ExternalOutput")

    with tile.TileContext(nc) as tc:
        from concourse import library_config
        nc.gpsimd.load_library(library_config.proxy)
        with (
            tc.tile_pool(name="persist", bufs=1) as persist,
            tc.tile_pool(name="attn", bufs=6) as apool,
            tc.tile_pool(name="norm", bufs=2) as npool,
            tc.tile_pool(name="small", bufs=4) as small,
            tc.tile_pool(name="y", bufs=3) as ypool,
            tc.tile_pool(name="spsum", bufs=2, space="PSUM") as spool,
            tc.tile_pool(name="avpsum", bufs=4, space="PSUM") as avpool,
        ):
            # ---- weights ----
            wq_sb = persist.tile([128, 4, 2 * HD], bf16, tag="wq")
            wk_sb = persist.tile([128, 4, 2 * HD], bf16, tag="wk")
            wv_sb = persist.tile([128, 4, 2 * HD], bf16, tag="wv")
            for ctr in range(4):
                nc.sync.dma_start(wq_sb[:, ctr, :], wq_d[ctr * 128:(ctr + 1) * 128, :])
                nc.sync.dma_start(wk_sb[:, ctr, :], wk_d[ctr * 128:(ctr + 1) * 128, :])
                nc.sync.dma_start(wv_sb[:, ctr, :], wv_d[ctr * 128:(ctr + 1) * 128, :])
            wp_sb = persist.tile([128, C], bf16, tag="wp")
            nc.sync.dma_start(wp_sb[:, :], wp_d[:, :])
            ones64 = persist.tile([1, HD], f32, tag="ones64")
            nc.vector.memset(ones64[:, :], 1.0)

            # ---- persistent per-row-group tiles ----
            xsb = [persist.tile([128, 4, 512], bf16, tag=f"xsb{rg}", name=f"xsb{rg}") for rg in range(RG)]
            qTt = [persist.tile([128, 512], bf16, tag=f"qT{rg}", name=f"qT{rg}") for rg in range(RG)]
            kTt = [persist.tile([128, 512], bf16, tag=f"kT{rg}", name=f"kT{rg}") for rg in range(RG)]
            vt = [persist.tile([128, 2, 4, HD + 1], bf16, tag=f"v{rg}", name=f"v{rg}") for rg in range(RG)]

            def prologue(rg):
                # DMA x[b].T chunk: [512 C x 512 rows] -> 4 partition chunks
                for ctr in range(4):
                    nc.sync.dma_start(
                        xsb[rg][:, ctr, :],
                        xt_d[ctr * 128:(ctr + 1) * 128, rg * 512:(rg + 1) * 512],
                    )
                # qT/kT: [c_out(2 heads x 64) x rows]  = w.T @ x.T-chunk
                ps_q = avpool.tile([128, 512], f32, tag="av")
                ps_k = avpool.tile([128, 512], f32, tag="av")
                for ctr in range(4):
                    nc.tensor.matmul(
                        ps_q[:, :], wq_sb[:, ctr, :], xsb[rg][:, ctr, :],
                        start=(ctr == 0), stop=(ctr == 3),
                    )
                for ctr in range(4):
                    nc.tensor.matmul(
                        ps_k[:, :], wk_sb[:, ctr, :], xsb[rg][:, ctr, :],
                        start=(ctr == 0), stop=(ctr == 3),
                    )
                nc.scalar.copy(qTt[rg][:, :], ps_q[:, :])
                nc.scalar.copy(kTt[rg][:, :], ps_k[:, :])
                # V natural layout: [rows x c_out]; rows on partitions
                ps_v = avpool.tile([128, 4, 128], f32, tag="av")
                for rcl in range(4):
                    for ctr in range(4):
                        nc.tensor.matmul(
                            ps_v[:, rcl, :],
                            xsb[rg][:, ctr, rcl * 128:(rcl + 1) * 128],
                            wv_sb[:, ctr, :],
                            start=(ctr == 0), stop=(ctr == 3),
                        )
                # scatter into v tiles: [128k x (h, rcl, d)] ; ones column
                nc.scalar.copy(
                    vt[rg][:, :, :, 0:HD],
                    ps_v.rearrange("p rcl (h d) -> p h rcl d", h=2),
                )
                nc.vector.memset(vt[rg][:, :, :, HD:HD + 1], 1.0)

            def qk_exp(qt, kc, use_act):
                rgk, kcl = divmod(kc, 4)
                sco = spool.tile([128, 2, 512], f32, tag="scores")
                # scores.T [k x q], two heads row-packed (K=64 each)
                nc.tensor.matmul(
                    sco[:, 0, :],
                    kTt[rgk][0:HD, kcl * 128:(kcl + 1) * 128],
                    qTt[qt][0:HD, :],
                    start=True, stop=True, skip_group_check=True,
                )
                nc.tensor.matmul(
                    sco[:, 1, :],
                    kTt[rgk][HD:2 * HD, kcl * 128:(kcl + 1) * 128],
                    qTt[qt][HD:2 * HD, :],
                    start=True, stop=True, skip_group_check=True,
                )
                att = apool.tile([128, 2, 512], bf16, tag="attn")
                if use_act:
                    nc.scalar.activation(att[:, :, :], sco[:, :, :], Exp, scale=SCALE)
                else:
                    nc.vector.tensor_scalar(
                        att.bitcast(i16)[:, :, :], sco[:, :, :], SCHR_A, SCHR_B,
                        op0=mybir.AluOpType.mult, op1=mybir.AluOpType.add,
                    )
                return att

            def av_acc(qt, kc, att, av0, av1, first, last):
                rgk, kcl = divmod(kc, 4)
                nc.tensor.matmul(
                    av0[:, :], vt[rgk][:, 0, kcl, :], att[:, 0, :],
                    start=first, stop=last, skip_group_check=True,
                )
                nc.tensor.matmul(
                    av1[:, :], vt[rgk][:, 1, kcl, :], att[:, 1, :],
                    start=first, stop=last, skip_group_check=True,
                )

            def post_stage1(qt, av0, av1):
                # move AV out of PSUM on the ScalarE (frees the av psum slots)
                avsb = npool.tile([128, 2, 512], f32, tag="avsb", name=f"avsb{qt}")
                nc.scalar.copy(avsb[0:HD + 1, 0, :], av0[:, :])
                nc.scalar.copy(avsb[0:HD + 1, 1, :], av1[:, :])
                return avsb

            def post_stage1b(qt, avsb):
                rec0 = small.tile([1, 512], f32, tag="rec0", name=f"rec0_{qt}")
                rec1 = small.tile([1, 512], f32, tag="rec1", name=f"rec1_{qt}")
                nc.vector.reciprocal(rec0[:, :], avsb[HD:HD + 1, 0, :])
                nc.vector.reciprocal(rec1[:, :], avsb[HD:HD + 1, 1, :])
                return avsb, rec0, rec1

            def post_stage2(qt, avsb, rec0, rec1):
                rbc0 = small.tile([HD, 512], f32, tag="rbc0", name=f"rbc0_{qt}")
                rbc1 = small.tile([HD, 512], f32, tag="rbc1", name=f"rbc1_{qt}")
                nc.gpsimd.partition_broadcast(rbc0[:, :], rec0[:, :])
                nc.gpsimd.partition_broadcast(rbc1[:, :], rec1[:, :])
                # normalized attention output, both heads stacked [128 x 512q]
                avn = npool.tile([128, 512], bf16, tag="avn", name=f"avn{qt}")
                nc.gpsimd.tensor_mul(avn[0:HD, :], avsb[0:HD, 0, :], rbc0[:, :])
                nc.gpsimd.tensor_mul(avn[HD:2 * HD, :], avsb[0:HD, 1, :], rbc1[:, :])
                return avn

            def post_stage3(qt, avn):
                for qc in range(4):
                    ps_y = avpool.tile([128, 512], f32, tag="av", name=f"psy{qt}_{qc}")
                    nc.tensor.matmul(
                        ps_y[:, :], avn[:, qc * 128:(qc + 1) * 128],
                        wp_sb[:, :], start=True, stop=True,
                        skip_group_check=True,
                    )
                    y_sb = ypool.tile([128, 512], bf16, tag="y", name=f"y{qt}_{qc}")
                    nc.scalar.copy(y_sb[:, :], ps_y[:, :])
                    nc.sync.dma_start(
                        out_d[qt * 512 + qc * 128: qt * 512 + (qc + 1) * 128, :],
                        y_sb[:, :],
                    )

            # ---- emission: weave prologue with qt0's k-sweep ----
            def emit_body():
                avs = {}
                pend = {}
                stage_q = []

                def push(qt, kc, use_act):
                    att = qk_exp(qt, kc, use_act)
                    if pend.get(qt) is not None:
                        pkc, patt = pend[qt]
                        av_acc(qt, pkc, patt, *avs[qt], pkc == 0, pkc == KC - 1)
                    pend[qt] = (kc, att)

                def drain_av(qt):
                    pkc, patt = pend.pop(qt)
                    av_acc(qt, pkc, patt, *avs[qt], pkc == 0, pkc == KC - 1)

                for rg in range(RG):
                    prologue(rg)
                for pair in range(QT // 2):
                    qtA, qtB = 2 * pair, 2 * pair + 1
                    avs[qtA] = (avpool.tile([HD + 1, 512], f32, tag="av", name=f"av0_t{qtA}"),
                                avpool.tile([HD + 1, 512], f32, tag="av", name=f"av1_t{qtA}"))
                    avs[qtB] = (avpool.tile([HD + 1, 512], f32, tag="av", name=f"av0_t{qtB}"),
                                avpool.tile([HD + 1, 512], f32, tag="av", name=f"av1_t{qtB}"))
                    for kc in range(KC):
                        push(qtA, kc, True)
                        push(qtB, kc, False)
                        if kc == 2 and stage_q:
                            stage_q.pop(0)()
                        if kc == 6 and stage_q:
                            stage_q.pop(0)()
                    drain_av(qtA)
                    drain_av(qtB)
                    import os as _os
                    if _os.environ.get("KERNEL_NOPOST"):
                        for qt in (qtA, qtB):
                            avsb = npool.tile([128, 2, 512], f32, tag="avsb", name=f"avsb{qt}")
                            a0, a1 = avs.pop(qt)
                            nc.scalar.copy(avsb[0:HD + 1, 0, :], a0[:, :])
                            nc.scalar.copy(avsb[0:HD + 1, 1, :], a1[:, :])
                            nc.sync.dma_start(out_d[qt * 512:qt * 512 + 128, :],
                                              avsb[:, 0, :])
                        continue
                    sbA = post_stage1(qtA, *avs.pop(qtA))
                    sbB = post_stage1(qtB, *avs.pop(qtB))
                    # previous pair's projection now that av slots cycled
                    if stage_q:
                        stage_q.pop(0)()

                    def mk1b(qa, sa, qb, sb_):
                        def s1b():
                            ca = post_stage1b(qa, sa)
                            cb = post_stage1b(qb, sb_)

                            def s2():
                                avn_a = post_stage2(qa, *ca)
                                avn_b = post_stage2(qb, *cb)
                                stage_q.append(lambda: (post_stage3(qa, avn_a),
                                                        post_stage3(qb, avn_b)))
                            stage_q.insert(0, s2)
                        return s1b

                    stage_q.append(mk1b(qtA, sbA, qtB, sbB))
                while stage_q:
                    stage_q.pop(0)()

            if loop_reps:
                with tc.For_i(0, loop_reps, 1):
                    emit_body()
            else:
                emit_body()

    nc.compile()
    return nc


def _get_nc():
    if "nc" not in _CACHE:
        _CACHE["nc"] = _build_bass()
    return _CACHE["nc"]


def _make_in_maps(x, w_qkv, w_proj):
    bf = ml_dtypes.bfloat16
    in_maps = []
    for core in range(NCORES):
        b, j = divmod(core, 4)
        xt = np.ascontiguousarray(x[b].T).astype(bf)            # [C, N]
        wq = np.ascontiguousarray(w_qkv[:, 128 * j:128 * j + 128]).astype(bf)
        wk = np.ascontiguousarray(w_qkv[:, C + 128 * j:C + 128 * j + 128]).astype(bf)
        wv = np.ascontiguousarray(w_qkv[:, 2 * C + 128 * j:2 * C + 128 * j + 128]).astype(bf)
        wp = np.ascontiguousarray(w_proj[128 * j:128 * j + 128, :]).astype(bf)
        in_maps.append({"xt": xt, "wq": wq, "wk": wk, "wv": wv, "wp": wp})
    return in_maps


def _run(x, w_qkv, w_proj, b_proj, trace=False):
    from concourse.bass_utils import run_bass_kernel_spmd

    nc = _get_nc()
    in_maps = _make_in_maps(x, w_qkv, w_proj)
    res = run_bass_kernel_spmd(nc, in_maps, core_ids=list(range(NCORES)), trace=trace)
    out = np.zeros((B, N, C), dtype=np.float32)
    for core in range(NCORES):
        b = core // 4
        out[b] += res.results[core]["out"].astype(np.float32)
    out += b_proj.astype(np.float32)
    return out, res


def kernel(x, w_qkv, w_proj, b_proj):
    x = np.asarray(x, dtype=np.float32)
    w_qkv = np.asarray(w_qkv, dtype=np.float32)
    w_proj = np.asarray(w_proj, dtype=np.float32)
    b_proj = np.asarray(b_proj, dtype=np.float32)
    out, _ = _run(x, w_qkv, w_proj, b_proj, trace=False)
    return out
